# revision 1
# baseline (speedup 1.0000x reference)
"""Trainium2 Bass kernel for complex depthwise batchnorm (training-mode stats).

Transposed data-parallel design, 8 NeuronCores, batch N split across cores.

Host side: each core's shard [NS, D] (NS=2048 rows, D=C*F=2056 cols) is cast
to bf16 and TRANSPOSED to [D_pad=2176, NS] so the (c,f) axis lives on SBUF
partitions (17 chunks of 128) and the batch axis is the free dim. Per-(c,f)
stats are free-axis reductions (accum_out / tensor_reduce on DVE, Square+
accum on ACT, products on Pool), and the per-(c,f) coefficients are
per-partition [128,1] scalars, so phase B uses DVE tensor_scalar (fast DVE
perf mode), ACT fused identity(x*scale+bias), and Pool tensor_tensor adds.

The 5x17 column stats are AllReduced in TWO halves (chunks 0..8 / 9..16) so
collective latency hides under phase A's tail and the first half's phase B.
All tensor_scalar ops write to scratch, never in place (in-place ts measured
~6x slower on HW). bf16 end-to-end keeps rel err ~3e-3 vs the 2e-2 gate.
"""

import numpy as np
import ml_dtypes

N, C, F = 16384, 8, 257
D = C * F            # 2056
P = 128
NCH = 17             # ceil(D / 128)
DP = NCH * P         # 2176 (zero-padded tail rows)
N_CORES = 8
NS = N // N_CORES    # 2048 (free dim per core)
EPS = 1e-6
DELTA_MAX = 1e8
INV_N = 1.0 / N

H1 = list(range(0, 9))       # first all-reduce half
H2 = list(range(9, NCH))     # second half
NQ = 5                       # stat quantities per chunk

# engine assignment per chunk
XI2_ACT = {1, 3, 5, 7, 9, 11, 13, 15}      # sum(xi^2) on ACT, else DVE
CROSS_POOL = {c for c in range(NCH) if c % 3 != 0}  # cross mult Pool, else DVE
T3_ACT = {0, 2, 4, 6, 8, 10, 12, 14, 16}   # phase-B t3 on ACT, else DVE
YI_DVE = {0, 3, 6, 9, 12, 15}              # phase-B yi-add on DVE, else Pool

_CACHE = {}

# stats column layout: half-1 block [0:45] (q*9 + idx), half-2 [45:85]
def st_col(q, c):
    if c < 9:
        return q * 9 + c
    return 45 + q * 8 + (c - 9)


def _build():
    import concourse.bacc as bacc
    import concourse.tile as tile
    import concourse.mybir as mybir

    f32 = mybir.dt.float32
    bf16 = mybir.dt.bfloat16
    Alu = mybir.AluOpType
    Act = mybir.ActivationFunctionType
    Ax = mybir.AxisListType

    nc = bacc.Bacc("TRN2", target_bir_lowering=False, debug=False,
                   num_devices=N_CORES)

    xrt = nc.dram_tensor("xrt", [DP, NS], bf16, kind="ExternalInput").ap()
    xit = nc.dram_tensor("xit", [DP, NS], bf16, kind="ExternalInput").ap()
    # wp columns: 5 quantities x 17 chunk-cols in st_col layout
    wp = nc.dram_tensor("wp", [P, 5 * NCH], f32, kind="ExternalInput").ap()
    yrt = nc.dram_tensor("yrt", [DP, NS], bf16, kind="ExternalOutput").ap()
    yit = nc.dram_tensor("yit", [DP, NS], bf16, kind="ExternalOutput").ap()

    with tile.TileContext(nc) as tc:
        with (
            tc.tile_pool(name="keep", bufs=1) as keep,
            tc.tile_pool(name="crp", bufs=2) as crp,
            tc.tile_pool(name="tb", bufs=2) as tb,
            tc.tile_pool(name="co", bufs=6) as cop,
            tc.tile_pool(name="dram", bufs=1, space="DRAM") as dram,
        ):
            V = nc.vector
            S = nc.scalar
            G = nc.gpsimd

            wpt = keep.tile([P, 5 * NCH], f32, name="wpt")
            nc.sync.dma_start(out=wpt[:], in_=wp[:, :])

            st = keep.tile([P, NQ * NCH], f32, name="st")
            # shared garbage-output tiles for accum ops (per-engine, WAW on
            # the same in-order engine costs nothing)
            dump_v = keep.tile([P, NS], bf16, name="dump_v")
            dump_a = keep.tile([P, NS], bf16, name="dump_a")

            cc_in1 = dram.tile([P, NQ * 9], f32, name="cc_in1")
            cc_out1 = dram.tile([P, NQ * 9], f32, name="cc_out1",
                                addr_space="Shared")
            cc_in2 = dram.tile([P, NQ * 8], f32, name="cc_in2")
            cc_out2 = dram.tile([P, NQ * 8], f32, name="cc_out2",
                                addr_space="Shared")

            xr_c, xi_c = [], []

            def phase_a_chunk(c):
                xt = keep.tile([P, NS], bf16, name=f"xr{c}")
                nc.sync.dma_start(out=xt[:], in_=xrt[c * P:(c + 1) * P, :])
                yt = keep.tile([P, NS], bf16, name=f"xi{c}")
                nc.sync.dma_start(out=yt[:], in_=xit[c * P:(c + 1) * P, :])
                xr_c.append(xt)
                xi_c.append(yt)

                # sum(xr^2): ACT square + accumulate
                S.activation(dump_a[:], xt[:], Act.Square,
                             accum_out=st[:, st_col(2, c):st_col(2, c) + 1])
                # sum(xi^2)
                if c in XI2_ACT:
                    S.activation(dump_a[:], yt[:], Act.Square,
                                 accum_out=st[:, st_col(3, c):st_col(3, c) + 1])
                else:
                    V.tensor_tensor(dump_v[:], yt[:], yt[:], Alu.mult)
                    V.tensor_scalar(dump_v[:], dump_v[:], 1.0, 0.0, Alu.mult,
                                    Alu.add,
                                    accum_out=st[:, st_col(3, c):st_col(3, c) + 1])
                # sum(xr*xi)
                if c in CROSS_POOL:
                    cr = crp.tile([P, NS], bf16, tag="cr", name=f"cr{c}")
                    G.tensor_tensor(cr[:], xt[:], yt[:], Alu.mult)
                    V.tensor_scalar(dump_v[:], cr[:], 1.0, 0.0, Alu.mult,
                                    Alu.add,
                                    accum_out=st[:, st_col(4, c):st_col(4, c) + 1])
                else:
                    V.tensor_tensor(dump_v[:], xt[:], yt[:], Alu.mult)
                    V.tensor_scalar(dump_v[:], dump_v[:], 1.0, 0.0, Alu.mult,
                                    Alu.add,
                                    accum_out=st[:, st_col(4, c):st_col(4, c) + 1])
                # plain sums: A/B two reduce flavors to compare on HW
                V.tensor_scalar(dump_v[:], xt[:], 1.0, 0.0, Alu.mult, Alu.add,
                                accum_out=st[:, st_col(0, c):st_col(0, c) + 1])
                V.tensor_reduce(st[:, st_col(1, c):st_col(1, c) + 1], yt[:],
                                Ax.X, Alu.add)

            # coefficient tiles, one column per chunk
            zrr = keep.tile([P, NCH], f32, name="zrr")
            zri = keep.tile([P, NCH], f32, name="zri")
            zir = keep.tile([P, NCH], f32, name="zir")
            zii = keep.tile([P, NCH], f32, name="zii")
            brp = keep.tile([P, NCH], f32, name="brp")
            bip = keep.tile([P, NCH], f32, name="bip")

            def coeff_math(h, gt, w):
                """gt: all-reduced [P, 5*nc_] sums (q-major); w: same layout
                params; writes coeff columns [P, lo:hi]."""
                nc_ = len(h)
                lo, hi = h[0], h[-1] + 1
                cs = slice(lo, hi)

                def q(t, i):
                    return t[:, i * nc_:(i + 1) * nc_]

                def stile(name):
                    # [P, <=9] fp32 tiles are 36B/partition: keep them all
                    return keep.tile([P, nc_], f32, name=f"{name}_{lo}")

                mr = stile("mr")
                V.tensor_scalar_mul(mr[:], q(gt, 0), INV_N)
                mi = stile("mi")
                V.tensor_scalar_mul(mi[:], q(gt, 1), INV_N)

                mr2 = stile("mr2")
                V.tensor_tensor(mr2[:], mr[:], mr[:], Alu.mult)
                mi2 = stile("mi2")
                V.tensor_tensor(mi2[:], mi[:], mi[:], Alu.mult)
                mri = stile("mri")
                V.tensor_tensor(mri[:], mr[:], mi[:], Alu.mult)

                vrr = stile("vrr")
                V.scalar_tensor_tensor(vrr[:], q(gt, 2), INV_N, mr2[:],
                                       Alu.mult, Alu.subtract)
                vii = stile("vii")
                V.scalar_tensor_tensor(vii[:], q(gt, 3), INV_N, mi2[:],
                                       Alu.mult, Alu.subtract)
                vri = stile("vri")
                V.scalar_tensor_tensor(vri[:], q(gt, 4), INV_N, mri[:],
                                       Alu.mult, Alu.subtract)

                tau = stile("tau")
                V.tensor_tensor(tau[:], vrr[:], vii[:], Alu.add)
                dl = stile("dl")
                V.tensor_tensor(dl[:], vrr[:], vii[:], Alu.mult)
                vri2 = stile("vri2")
                V.tensor_tensor(vri2[:], vri[:], vri[:], Alu.mult)
                delta = stile("delta")
                V.tensor_tensor(delta[:], dl[:], vri2[:], Alu.subtract)
                V.tensor_scalar(delta[:], delta[:], EPS, DELTA_MAX,
                                Alu.max, Alu.min)

                s_t = stile("s_t")
                S.activation(s_t[:], delta[:], Act.Sqrt)
                targ = stile("targ")
                V.scalar_tensor_tensor(targ[:], s_t[:], 2.0, tau[:],
                                       Alu.mult, Alu.add)
                t_t = stile("t_t")
                S.activation(t_t[:], targ[:], Act.Sqrt)
                stt_ = stile("stt")
                V.tensor_tensor(stt_[:], s_t[:], t_t[:], Alu.mult)
                rst = stile("rst")
                V.reciprocal(rst[:], stt_[:])

                a1 = stile("a1")
                V.tensor_tensor(a1[:], s_t[:], vii[:], Alu.add)
                urr = stile("urr")
                V.tensor_tensor(urr[:], a1[:], rst[:], Alu.mult)
                a2 = stile("a2")
                V.tensor_tensor(a2[:], s_t[:], vrr[:], Alu.add)
                uii = stile("uii")
                V.tensor_tensor(uii[:], a2[:], rst[:], Alu.mult)
                uri = stile("uri")
                V.scalar_tensor_tensor(uri[:], vri[:], -1.0, rst[:],
                                       Alu.mult, Alu.mult)

                def mix(zt, wa, ua, wb, ub, nm):
                    g1 = stile(nm + "g1")
                    V.tensor_tensor(g1[:], wa, ua[:], Alu.mult)
                    g2 = stile(nm + "g2")
                    V.tensor_tensor(g2[:], wb, ub[:], Alu.mult)
                    V.tensor_tensor(zt[:, cs], g1[:], g2[:], Alu.add)

                mix(zrr, q(w, 0), urr, q(w, 1), uri, "zrr")
                mix(zri, q(w, 0), uri, q(w, 1), uii, "zri")
                mix(zir, q(w, 1), urr, q(w, 2), uri, "zir")
                mix(zii, q(w, 1), uri, q(w, 2), uii, "zii")

                def bias(bt, b0, za, zb, nm):
                    h1 = stile(nm + "h1")
                    V.tensor_tensor(h1[:], za[:, cs], mr[:], Alu.mult)
                    h2 = stile(nm + "h2")
                    V.tensor_tensor(h2[:], zb[:, cs], mi[:], Alu.mult)
                    h3 = stile(nm + "h3")
                    V.tensor_tensor(h3[:], h1[:], h2[:], Alu.add)
                    V.tensor_tensor(bt[:, cs], b0, h3[:], Alu.subtract)

                bias(brp, q(w, 3), zrr, zri, "brp")
                bias(bip, q(w, 4), zir, zii, "bip")

            def phase_b_chunk(c):
                xt, yt = xr_c[c], xi_c[c]
                cs = slice(c, c + 1)
                # t1 = xr*Zrr + br'
                t1 = tb.tile([P, NS], bf16, tag="t1", name=f"t1_{c}")
                if c in T3_ACT:
                    V.tensor_scalar(t1[:], xt[:], zrr[:, cs], brp[:, cs],
                                    Alu.mult, Alu.add)
                else:
                    S.activation(t1[:], xt[:], Act.Identity,
                                 bias=brp[:, cs], scale=zrr[:, cs])
                # t2 = xi*Zri ; yr = t1 + t2 (in-place tt into t2 is fine)
                t2 = tb.tile([P, NS], bf16, tag="t2", name=f"t2_{c}")
                V.tensor_scalar(t2[:], yt[:], zri[:, cs], None, Alu.mult)
                V.tensor_tensor(t2[:], t1[:], t2[:], Alu.add)
                nc.sync.dma_start(out=yrt[c * P:(c + 1) * P, :], in_=t2[:])
                # t3 = xi*Zii + bi'
                t3 = tb.tile([P, NS], bf16, tag="t3", name=f"t3_{c}")
                if c in T3_ACT:
                    S.activation(t3[:], yt[:], Act.Identity,
                                 bias=bip[:, cs], scale=zii[:, cs])
                else:
                    V.tensor_scalar(t3[:], yt[:], zii[:, cs], bip[:, cs],
                                    Alu.mult, Alu.add)
                # t4 = xr*Zir ; yi = t4 + t3 (in-place tt into t4)
                t4 = tb.tile([P, NS], bf16, tag="t4", name=f"t4_{c}")
                V.tensor_scalar(t4[:], xt[:], zir[:, cs], None, Alu.mult)
                if c in YI_DVE:
                    V.tensor_tensor(t4[:], t4[:], t3[:], Alu.add)
                else:
                    G.tensor_tensor(t4[:], t4[:], t3[:], Alu.add)
                nc.sync.dma_start(out=yit[c * P:(c + 1) * P, :], in_=t4[:])

            # ---------------- schedule ----------------
            for c in H1:
                phase_a_chunk(c)
            nc.sync.dma_start(out=cc_in1[:, :], in_=st[:, 0:NQ * 9])
            G.collective_compute(
                "AllReduce", Alu.add,
                replica_groups=[list(range(N_CORES))],
                ins=[cc_in1[:].opt()], outs=[cc_out1[:].opt()])

            for c in H2:
                phase_a_chunk(c)
            nc.sync.dma_start(out=cc_in2[:, :], in_=st[:, NQ * 9:])
            G.collective_compute(
                "AllReduce", Alu.add,
                replica_groups=[list(range(N_CORES))],
                ins=[cc_in2[:].opt()], outs=[cc_out2[:].opt()])

            gt1 = keep.tile([P, NQ * 9], f32, name="gt1")
            nc.sync.dma_start(out=gt1[:], in_=cc_out1[:, :])
            coeff_math(H1, gt1, wpt[:, 0:NQ * 9])
            for c in H1:
                phase_b_chunk(c)

            gt2 = keep.tile([P, NQ * 8], f32, name="gt2")
            nc.sync.dma_start(out=gt2[:], in_=cc_out2[:, :])
            coeff_math(H2, gt2, wpt[:, NQ * 9:])
            for c in H2:
                phase_b_chunk(c)

    nc.compile()
    return nc


def get_nc():
    if "nc" not in _CACHE:
        _CACHE["nc"] = _build()
    return _CACHE["nc"]


def make_in_maps(xr, xi, Wrr, Wri, Wii, Br, Bi):
    bf = ml_dtypes.bfloat16
    xr2 = np.asarray(xr).reshape(N, D)
    xi2 = np.asarray(xi).reshape(N, D)
    xr_bf = xr2.astype(bf)
    xi_bf = xi2.astype(bf)

    # params -> [P, 5*NCH] in the st_col layout (q-major inside each half)
    def to_cols(a):
        v = np.zeros(DP, dtype=np.float32)
        v[:D] = np.asarray(a).reshape(D)
        return v.reshape(NCH, P).T          # [P, NCH], col c = chunk c

    cols = [to_cols(Wrr), to_cols(Wri), to_cols(Wii), to_cols(Br), to_cols(Bi)]
    wp = np.zeros((P, 5 * NCH), dtype=np.float32)
    for q in range(5):
        wp[:, q * 9:(q + 1) * 9] = cols[q][:, 0:9]
        wp[:, 45 + q * 8:45 + (q + 1) * 8] = cols[q][:, 9:17]

    in_maps = []
    for r in range(N_CORES):
        xrt = np.zeros((DP, NS), dtype=bf)
        xrt[:D] = xr_bf[r * NS:(r + 1) * NS].T
        xit = np.zeros((DP, NS), dtype=bf)
        xit[:D] = xi_bf[r * NS:(r + 1) * NS].T
        in_maps.append({"xrt": xrt, "xit": xit, "wp": wp})
    return in_maps


def kernel(xr, xi, Wrr, Wri, Wii, Br, Bi):
    from concourse import bass_utils

    nc = get_nc()
    in_maps = make_in_maps(xr, xi, Wrr, Wri, Wii, Br, Bi)
    res = bass_utils.run_bass_kernel_spmd(nc, in_maps,
                                          core_ids=list(range(N_CORES)))
    yr = np.concatenate(
        [np.asarray(res.results[r]["yrt"])[:D].T for r in range(N_CORES)],
        axis=0).astype(np.float32)
    yi = np.concatenate(
        [np.asarray(res.results[r]["yit"])[:D].T for r in range(N_CORES)],
        axis=0).astype(np.float32)
    return yr.reshape(N, C, F), yi.reshape(N, C, F)



# revision 3
# speedup vs baseline: 1.4945x; 1.4945x over previous
"""Trainium2 Bass kernel for complex depthwise batchnorm (training-mode stats).

v2 design, 8 NeuronCores, batch N split across cores, transposed layout:
each core's shard [NS=2048, D=2056] is cast to bf16 and transposed host-side
to [DP=2176, NS] so (c,f) dims live on SBUF partitions (17 chunks of 128) and
batch is the free axis.

Per-chunk work:
  Phase A (stats):
    - DVE bn_stats x4 subtiles + bn_aggr per tensor -> per-core (mean, var)
      in ONE streaming pass each (replaces separate sum and sum-sq passes).
    - cross product xr*xi on Pool (tensor_tensor), summed by ACT
      (Identity activation with accum_out).
  AllReduce of [mean_r, mean_i, E[xr^2], E[xi^2], cross_sum] per (c,f),
  split into 4 chunk-groups (3,5,5,4) so the collective latency hides
  under phase A of later groups and phase B of earlier ones.
  Phase B (normalize+affine): runs on the otherwise-idle TensorEngine:
      yr = diag(Zrr) @ xr + diag(Zri) @ xi   (PSUM accumulate)
      yi = diag(Zir) @ xr + diag(Zii) @ xi
    with diag matrices built by DVE as ident * z_col (tensor_scalar).
    ACT (mostly) evacuates PSUM->SBUF bf16 fusing the per-partition bias.

Queues: bulk loads+stores on sync, collective staging on ACT, collective
results gathered via the Tensor queue (PE waits on them anyway), AR trigger
on GpSimd. bf16 end-to-end keeps rel err ~3e-3 vs the 2e-2 gate.
"""

import numpy as np
import ml_dtypes

N, C, F = 16384, 8, 257
D = C * F            # 2056
P = 128
NCH = 17             # ceil(D / 128)
DP = NCH * P         # 2176 (zero-padded tail rows)
N_CORES = 8
NS = N // N_CORES    # 2048 (free dim per core)
SUB = 512            # PSUM-bank-sized subtile of the batch axis
NSUB = NS // SUB     # 4
EPS = 1e-6
DELTA_MAX = 1e8
INV_N = 1.0 / N
INV_W = 1.0 / N_CORES

GROUPS = [[0, 1, 2], [3, 4, 5, 6, 7], [8, 9, 10, 11, 12], [13, 14, 15, 16]]
NQ = 5               # stat quantities per chunk: mr, mi, er, ei, cross

# chunks whose PSUM evacuation runs on DVE instead of ACT (load balance)
EVAC_DVE = {0, 2}

_CACHE = {}


def _build():
    import concourse.bacc as bacc
    import concourse.tile as tile
    import concourse.mybir as mybir

    f32 = mybir.dt.float32
    bf16 = mybir.dt.bfloat16
    Alu = mybir.AluOpType
    Act = mybir.ActivationFunctionType

    nc = bacc.Bacc("TRN2", target_bir_lowering=False, debug=False,
                   num_devices=N_CORES)

    xrt = nc.dram_tensor("xrt", [DP, NS], bf16, kind="ExternalInput").ap()
    xit = nc.dram_tensor("xit", [DP, NS], bf16, kind="ExternalInput").ap()
    # wp columns: 5 quantities x 17 chunk-cols, q-major (q*NCH + c)
    wp = nc.dram_tensor("wp", [P, 5 * NCH], f32, kind="ExternalInput").ap()
    ident = nc.dram_tensor("ident", [P, P], bf16, kind="ExternalInput").ap()
    yrt = nc.dram_tensor("yrt", [DP, NS], bf16, kind="ExternalOutput").ap()
    yit = nc.dram_tensor("yit", [DP, NS], bf16, kind="ExternalOutput").ap()

    with tile.TileContext(nc) as tc:
        with (
            tc.tile_pool(name="keep", bufs=1) as keep,
            tc.tile_pool(name="bs", bufs=4) as bsp,
            tc.tile_pool(name="crp", bufs=2) as crp,
            tc.tile_pool(name="dg", bufs=12) as dgp,
            tc.tile_pool(name="yo", bufs=3) as yop,
            tc.tile_pool(name="co", bufs=6) as cop,
            tc.tile_pool(name="ps", bufs=8, space="PSUM") as psp,
            tc.tile_pool(name="dram", bufs=1, space="DRAM") as dram,
        ):
            V = nc.vector
            S = nc.scalar
            G = nc.gpsimd
            T = nc.tensor
            SY = nc.sync

            wpt = keep.tile([P, 5 * NCH], f32, name="wpt")
            SY.dma_start(out=wpt[:], in_=wp[:, :])
            idt = keep.tile([P, P], bf16, name="idt")
            SY.dma_start(out=idt[:], in_=ident[:, :])

            # garbage-output tile for ACT accum ops
            dump_a = keep.tile([P, NS], bf16, name="dump_a")

            # ---------------- all input loads up front (sync queue) -----
            xr_c, xi_c = [None] * NCH, [None] * NCH
            for g in GROUPS:
                for c in g:
                    xt = keep.tile([P, NS], bf16, name=f"xr{c}")
                    SY.dma_start(out=xt[:], in_=xrt[c * P:(c + 1) * P, :])
                    yt = keep.tile([P, NS], bf16, name=f"xi{c}")
                    SY.dma_start(out=yt[:], in_=xit[c * P:(c + 1) * P, :])
                    xr_c[c] = xt
                    xi_c[c] = yt

            # per-group staging/result tiles
            cc_sb, mv_r, mv_i, gts = [], [], [], []
            cc_in, cc_out = [], []
            for gi, g in enumerate(GROUPS):
                ng = len(g)
                cc_sb.append(keep.tile([P, NQ * ng], f32, name=f"ccsb{gi}"))
                mv_r.append(keep.tile([P, 2 * ng], f32, name=f"mvr{gi}"))
                mv_i.append(keep.tile([P, 2 * ng], f32, name=f"mvi{gi}"))
                gts.append(keep.tile([P, NQ * ng], f32, name=f"gt{gi}"))
                cc_in.append(dram.tile([P, NQ * ng], f32, name=f"ccin{gi}"))
                cc_out.append(dram.tile([P, NQ * ng], f32, name=f"ccout{gi}",
                                        addr_space="Shared"))

            # coefficient tiles, one column per chunk
            zrr = keep.tile([P, NCH], f32, name="zrr")
            zri = keep.tile([P, NCH], f32, name="zri")
            zir = keep.tile([P, NCH], f32, name="zir")
            zii = keep.tile([P, NCH], f32, name="zii")
            brp = keep.tile([P, NCH], f32, name="brp")
            bip = keep.tile([P, NCH], f32, name="bip")

            def phase_a_chunk(gi, j, c):
                ng = len(GROUPS[gi])
                xt, yt = xr_c[c], xi_c[c]
                # xr stats: 4 bn_stats subtiles + aggregate
                bsr = bsp.tile([P, NSUB, 6], f32, tag="bsr", name=f"bsr{c}")
                for s in range(NSUB):
                    V.bn_stats(out=bsr[:, s, :],
                               in_=xt[:, s * SUB:(s + 1) * SUB])
                V.bn_aggr(out=mv_r[gi][:, j::ng], in_=bsr[:])
                # xi stats
                bsi = bsp.tile([P, NSUB, 6], f32, tag="bsi", name=f"bsi{c}")
                for s in range(NSUB):
                    V.bn_stats(out=bsi[:, s, :],
                               in_=yt[:, s * SUB:(s + 1) * SUB])
                V.bn_aggr(out=mv_i[gi][:, j::ng], in_=bsi[:])
                # cross product on Pool, summed on ACT
                cr = crp.tile([P, NS], bf16, tag="cr", name=f"cr{c}")
                G.tensor_tensor(cr[:], xt[:], yt[:], Alu.mult)
                S.activation(dump_a[:], cr[:], Act.Identity,
                             accum_out=cc_sb[gi][:, 4 * ng + j:4 * ng + j + 1])

            def stage_group(gi):
                """mean/var -> AllReduce payload; stage + trigger + gather."""
                ng = len(GROUPS[gi])
                cs, mr, mi = cc_sb[gi], mv_r[gi], mv_i[gi]
                # q0 = mean_r, q1 = mean_i (summed across cores -> /8 later)
                V.tensor_copy(cs[:, 0:ng], mr[:, 0:ng])
                V.tensor_copy(cs[:, ng:2 * ng], mi[:, 0:ng])
                # q2 = E[xr^2] = var_r + mean_r^2 ; q3 likewise for xi
                tm = cop.tile([P, ng], f32, tag="tm", name=f"tm{gi}")
                V.tensor_tensor(tm[:], mr[:, 0:ng], mr[:, 0:ng], Alu.mult)
                V.tensor_tensor(cs[:, 2 * ng:3 * ng], tm[:], mr[:, ng:2 * ng],
                                Alu.add)
                tm2 = cop.tile([P, ng], f32, tag="tm2", name=f"tm2{gi}")
                V.tensor_tensor(tm2[:], mi[:, 0:ng], mi[:, 0:ng], Alu.mult)
                V.tensor_tensor(cs[:, 3 * ng:4 * ng], tm2[:], mi[:, ng:2 * ng],
                                Alu.add)
                # q4 (cross sums) already accumulated in place by ACT
                S.dma_start(out=cc_in[gi][:, :], in_=cs[:])
                G.collective_compute(
                    "AllReduce", Alu.add,
                    replica_groups=[list(range(N_CORES))],
                    ins=[cc_in[gi][:].opt()], outs=[cc_out[gi][:].opt()])
                SY.dma_start(out=gts[gi][:], in_=cc_out[gi][:, :])

            def coeff_math(gi):
                g = GROUPS[gi]
                ng = len(g)
                lo, hi = g[0], g[-1] + 1
                cs = slice(lo, hi)
                gt = gts[gi]

                def q(t, i):
                    return t[:, i * ng:(i + 1) * ng]

                def w(i):
                    return wpt[:, i * NCH + lo:i * NCH + hi]

                def stile(name):
                    return keep.tile([P, ng], f32, name=f"{name}_{lo}")

                mr = stile("mr")
                V.tensor_scalar_mul(mr[:], q(gt, 0), INV_W)
                mi = stile("mi")
                V.tensor_scalar_mul(mi[:], q(gt, 1), INV_W)

                mr2 = stile("mr2")
                V.tensor_tensor(mr2[:], mr[:], mr[:], Alu.mult)
                mi2 = stile("mi2")
                V.tensor_tensor(mi2[:], mi[:], mi[:], Alu.mult)
                mri = stile("mri")
                V.tensor_tensor(mri[:], mr[:], mi[:], Alu.mult)

                vrr = stile("vrr")
                V.scalar_tensor_tensor(vrr[:], q(gt, 2), INV_W, mr2[:],
                                       Alu.mult, Alu.subtract)
                vii = stile("vii")
                V.scalar_tensor_tensor(vii[:], q(gt, 3), INV_W, mi2[:],
                                       Alu.mult, Alu.subtract)
                vri = stile("vri")
                V.scalar_tensor_tensor(vri[:], q(gt, 4), INV_N, mri[:],
                                       Alu.mult, Alu.subtract)

                tau = stile("tau")
                V.tensor_tensor(tau[:], vrr[:], vii[:], Alu.add)
                dl = stile("dl")
                V.tensor_tensor(dl[:], vrr[:], vii[:], Alu.mult)
                vri2 = stile("vri2")
                V.tensor_tensor(vri2[:], vri[:], vri[:], Alu.mult)
                delta = stile("delta")
                V.tensor_tensor(delta[:], dl[:], vri2[:], Alu.subtract)
                V.tensor_scalar(delta[:], delta[:], EPS, DELTA_MAX,
                                Alu.max, Alu.min)

                s_t = stile("s_t")
                S.activation(s_t[:], delta[:], Act.Sqrt)
                targ = stile("targ")
                V.scalar_tensor_tensor(targ[:], s_t[:], 2.0, tau[:],
                                       Alu.mult, Alu.add)
                t_t = stile("t_t")
                S.activation(t_t[:], targ[:], Act.Sqrt)
                stt_ = stile("stt")
                V.tensor_tensor(stt_[:], s_t[:], t_t[:], Alu.mult)
                rst = stile("rst")
                V.reciprocal(rst[:], stt_[:])

                a1 = stile("a1")
                V.tensor_tensor(a1[:], s_t[:], vii[:], Alu.add)
                urr = stile("urr")
                V.tensor_tensor(urr[:], a1[:], rst[:], Alu.mult)
                a2 = stile("a2")
                V.tensor_tensor(a2[:], s_t[:], vrr[:], Alu.add)
                uii = stile("uii")
                V.tensor_tensor(uii[:], a2[:], rst[:], Alu.mult)
                uri = stile("uri")
                V.scalar_tensor_tensor(uri[:], vri[:], -1.0, rst[:],
                                       Alu.mult, Alu.mult)

                def mix(zt, wa, ua, wb, ub, nm):
                    g1 = stile(nm + "g1")
                    V.tensor_tensor(g1[:], wa, ua[:], Alu.mult)
                    g2 = stile(nm + "g2")
                    V.tensor_tensor(g2[:], wb, ub[:], Alu.mult)
                    V.tensor_tensor(zt[:, cs], g1[:], g2[:], Alu.add)

                mix(zrr, w(0), urr, w(1), uri, "zrr")
                mix(zri, w(0), uri, w(1), uii, "zri")
                mix(zir, w(1), urr, w(2), uri, "zir")
                mix(zii, w(1), uri, w(2), uii, "zii")

                def bias(bt, b0, za, zb, nm):
                    h1 = stile(nm + "h1")
                    V.tensor_tensor(h1[:], za[:, cs], mr[:], Alu.mult)
                    h2 = stile(nm + "h2")
                    V.tensor_tensor(h2[:], zb[:, cs], mi[:], Alu.mult)
                    h3 = stile(nm + "h3")
                    V.tensor_tensor(h3[:], h1[:], h2[:], Alu.add)
                    V.tensor_tensor(bt[:, cs], b0, h3[:], Alu.subtract)

                bias(brp, w(3), zrr, zri, "brp")
                bias(bip, w(4), zir, zii, "bip")

            def phase_b_chunk(c):
                xt, yt = xr_c[c], xi_c[c]
                col = slice(c, c + 1)
                # build the 4 diagonal stationaries for this chunk (DVE)
                dgs = []
                for nm, zt in (("rr", zrr), ("ri", zri), ("ir", zir),
                               ("ii", zii)):
                    dg = dgp.tile([P, P], bf16, tag="dg", name=f"d{nm}{c}")
                    V.tensor_scalar(dg[:], idt[:], zt[:, col], None, Alu.mult)
                    dgs.append(dg)
                drr, dri, dir_, dii = dgs

                yro = yop.tile([P, NS], bf16, tag="yro", name=f"yro{c}")
                yio = yop.tile([P, NS], bf16, tag="yio", name=f"yio{c}")
                for h in range(2):
                    s0, s1 = 2 * h, 2 * h + 1
                    sl0 = slice(s0 * SUB, (s0 + 1) * SUB)
                    sl1 = slice(s1 * SUB, (s1 + 1) * SUB)
                    pr0 = psp.tile([P, SUB], f32, tag="ps", name=f"pr0_{c}{h}")
                    pr1 = psp.tile([P, SUB], f32, tag="ps", name=f"pr1_{c}{h}")
                    pi0 = psp.tile([P, SUB], f32, tag="ps", name=f"pi0_{c}{h}")
                    pi1 = psp.tile([P, SUB], f32, tag="ps", name=f"pi1_{c}{h}")
                    # yr = diag(Zrr) @ xr + diag(Zri) @ xi
                    T.matmul(pr0[:], drr[:], xt[:, sl0], start=True,
                             stop=False)
                    T.matmul(pr1[:], drr[:], xt[:, sl1], start=True,
                             stop=False)
                    T.matmul(pr0[:], dri[:], yt[:, sl0], start=False,
                             stop=True)
                    T.matmul(pr1[:], dri[:], yt[:, sl1], start=False,
                             stop=True)
                    # yi = diag(Zir) @ xr + diag(Zii) @ xi
                    T.matmul(pi0[:], dir_[:], xt[:, sl0], start=True,
                             stop=False)
                    T.matmul(pi1[:], dir_[:], xt[:, sl1], start=True,
                             stop=False)
                    T.matmul(pi0[:], dii[:], yt[:, sl0], start=False,
                             stop=True)
                    T.matmul(pi1[:], dii[:], yt[:, sl1], start=False,
                             stop=True)
                    # evacuate PSUM -> SBUF bf16, fusing per-partition bias
                    if c in EVAC_DVE:
                        V.tensor_scalar(yro[:, sl0], pr0[:], brp[:, col],
                                        None, Alu.add)
                        V.tensor_scalar(yro[:, sl1], pr1[:], brp[:, col],
                                        None, Alu.add)
                        V.tensor_scalar(yio[:, sl0], pi0[:], bip[:, col],
                                        None, Alu.add)
                        V.tensor_scalar(yio[:, sl1], pi1[:], bip[:, col],
                                        None, Alu.add)
                    else:
                        S.activation(yro[:, sl0], pr0[:], Act.Identity,
                                     bias=brp[:, col])
                        S.activation(yro[:, sl1], pr1[:], Act.Identity,
                                     bias=brp[:, col])
                        S.activation(yio[:, sl0], pi0[:], Act.Identity,
                                     bias=bip[:, col])
                        S.activation(yio[:, sl1], pi1[:], Act.Identity,
                                     bias=bip[:, col])
                SY.dma_start(out=yrt[c * P:(c + 1) * P, :], in_=yro[:])
                SY.dma_start(out=yit[c * P:(c + 1) * P, :], in_=yio[:])

            def finish_group(gi):
                coeff_math(gi)
                for c in GROUPS[gi]:
                    phase_b_chunk(c)

            # ---------------- schedule ----------------
            for gi, g in enumerate(GROUPS):
                for j, c in enumerate(g):
                    phase_a_chunk(gi, j, c)
                stage_group(gi)
                if gi >= 1:
                    finish_group(gi - 1)
            finish_group(len(GROUPS) - 2)
            finish_group(len(GROUPS) - 1)

    nc.compile()
    return nc


def get_nc():
    if "nc" not in _CACHE:
        _CACHE["nc"] = _build()
    return _CACHE["nc"]


def make_in_maps(xr, xi, Wrr, Wri, Wii, Br, Bi):
    bf = ml_dtypes.bfloat16
    xr2 = np.asarray(xr).reshape(N, D)
    xi2 = np.asarray(xi).reshape(N, D)
    xr_bf = xr2.astype(bf)
    xi_bf = xi2.astype(bf)

    # params -> [P, 5*NCH], q-major (q*NCH + chunk)
    def to_cols(a):
        v = np.zeros(DP, dtype=np.float32)
        v[:D] = np.asarray(a).reshape(D)
        return v.reshape(NCH, P).T          # [P, NCH], col c = chunk c

    wp = np.concatenate(
        [to_cols(a) for a in (Wrr, Wri, Wii, Br, Bi)], axis=1
    ).astype(np.float32)
    ident = np.eye(P, dtype=bf)

    in_maps = []
    for r in range(N_CORES):
        xrt = np.zeros((DP, NS), dtype=bf)
        xrt[:D] = xr_bf[r * NS:(r + 1) * NS].T
        xit = np.zeros((DP, NS), dtype=bf)
        xit[:D] = xi_bf[r * NS:(r + 1) * NS].T
        in_maps.append({"xrt": xrt, "xit": xit, "wp": wp, "ident": ident})
    return in_maps


def kernel(xr, xi, Wrr, Wri, Wii, Br, Bi):
    from concourse import bass_utils

    nc = get_nc()
    in_maps = make_in_maps(xr, xi, Wrr, Wri, Wii, Br, Bi)
    res = bass_utils.run_bass_kernel_spmd(nc, in_maps,
                                          core_ids=list(range(N_CORES)))
    yr = np.concatenate(
        [np.asarray(res.results[r]["yrt"])[:D].T for r in range(N_CORES)],
        axis=0).astype(np.float32)
    yi = np.concatenate(
        [np.asarray(res.results[r]["yit"])[:D].T for r in range(N_CORES)],
        axis=0).astype(np.float32)
    return yr.reshape(N, C, F), yi.reshape(N, C, F)
